# revision 1
# baseline (speedup 1.0000x reference)
"""Trainium2 Bass kernel for BasicLSTM (B=64, T=512, D=U=512).

Sharding: data-parallel over batch across 8 cores (8 rows/core), weights
replicated; the sequential time scan runs locally per core.

Per-core strategy (everything unit-major / "transposed", all-SBUF):
  Phase A: zx.T = Wk.T @ x.T + b computed directly in unit-major layout.
    x is loaded with fast contiguous DMAs, converted to bf16, transposed
    on-chip via the DMA xbar (dedicated queue), then used as the moving
    operand against stationary bf16 Wk tiles.  Bias is applied via the ACT
    per-partition bias during PSUM->SBUF copy-out.  The whole
    zx.T [128p, T*(16m*8b)] stays resident in SBUF as bf16 (16 MB).
  Phase B: 512-step scan with zero DMA.  Gate m-tiles are reordered
    [i,f,o,g] and the 16 m-tiles are processed in two halves, each into its
    own PSUM bank, so the elementwise tail of the first half overlaps the
    matmuls of the second:
      z.T[t] = sum_k Wr[k,m].T @ h.T[k]      (PE, bf16+FWL, 64 LDW+MM)
      psum += zx.T[t]                        (DVE, in place)
      i,f,o = sigmoid(psum), g = tanh(psum)  (ACT, reads PSUM)
      c' = f*c + i*g ; h' = o*tanh(c')       (DVE/ACT)
    h/c are split into per-half tiles; h is bf16 (feeds the next matmul),
    c stays fp32.  The final h is computed in fp32 and DMA'd out.
"""

import numpy as np

B, T, D, U = 64, 512, 512, 512
G = 4 * U            # gates
P = 128              # partitions
N_CORES = 8
B_LOC = B // N_CORES  # 8
KD = D // P          # 4 k-tiles for x@Wk
KU = U // P          # 4 k-tiles for h@Wr
M = G // P           # 16 m-tiles of gates
TC = 64              # timesteps per phase-A chunk
FB = M * B_LOC       # 128 free cols of z per step
HB = FB // 2         # 64 cols per half

# gate reordering: new m-tile order [i, f, o, g] -> original m-tile index
PERMM = list(range(8)) + [12, 13, 14, 15] + [8, 9, 10, 11]
# halves: half h holds m-tiles {4a + q : a in 0..3} for q in {2h, 2h+1}
HALF_MS = [[0, 4, 8, 12, 1, 5, 9, 13], [2, 6, 10, 14, 3, 7, 11, 15]]

_CACHE = {}


def _build(time_steps=T):
    import concourse.bacc as bacc
    import concourse.tile as tile
    import concourse.mybir as mybir
    from bass_rust import add_dep_helper

    f32 = mybir.dt.float32
    bf16 = mybir.dt.bfloat16
    AF = mybir.ActivationFunctionType

    nc = bacc.Bacc(
        "TRN2",
        target_bir_lowering=False,
        debug=False,
        enable_asserts=True,
        num_devices=N_CORES,
    )

    x_h = nc.dram_tensor("x", [B_LOC, T, D], f32, kind="ExternalInput")
    wk_h = nc.dram_tensor("Wk", [D, G], f32, kind="ExternalInput")
    wr_h = nc.dram_tensor("Wr", [U, G], f32, kind="ExternalInput")
    b_h = nc.dram_tensor("b", [G], f32, kind="ExternalInput")
    out_h = nc.dram_tensor("h_last", [B_LOC, U], f32, kind="ExternalOutput")

    x_ap = x_h.ap()

    def load_weight_bf16(dst, src_h, stage_pool):
        """[512, 2048] fp32 weight -> dst bf16 [128, 64*128] laid out as
        (k, new_m) tiles of [128, 128] with the [i,f,o,g] gate reorder."""
        for k in range(KD):
            st = stage_pool.tile([P, G], f32, name="wstage", tag="wstage")
            nc.gpsimd.dma_start(st[:], src_h.ap()[k * P:(k + 1) * P, :])
            for nm0, om0, w in ((0, 0, 8), (8, 12, 4), (12, 8, 4)):
                nc.vector.tensor_copy(
                    dst[:, (k * M + nm0) * P:(k * M + nm0 + w) * P],
                    st[:, om0 * P:(om0 + w) * P],
                )

    with tile.TileContext(nc) as tc:
        with (
            tc.tile_pool(name="persist", bufs=1) as persist_pool,
        ):
            # zx.T resident in SBUF: col = m*(T*8) + b*64 + t  (bf16, 128KB/par)
            # (phase A writes [128, 512] contiguous per (m, chunk); the scan
            #  reads a strided comb per step, which is free on DVE)
            zxT = persist_pool.tile([P, T * FB], bf16)
            zxT4 = zxT.rearrange("p (m b t) -> p m b t", m=M, b=B_LOC)
            b_sb = persist_pool.tile([P, M], f32)
            nc.sync.dma_start(b_sb[:], b_h.ap().rearrange("(m p) -> p m", p=P))

            # ---------------- Phase A: zx.T = Wk.T @ x.T + b ----------------
            with (
                tc.tile_pool(name="wk", bufs=1) as wk_pool,
                tc.tile_pool(name="stage", bufs=2) as stage_pool,
                tc.tile_pool(name="nat", bufs=2) as nat_pool,
                tc.tile_pool(name="xtb", bufs=2) as xtb_pool,
                tc.tile_pool(name="gemm_psum", bufs=4, space="PSUM") as gps_pool,
            ):
                wk_sb = wk_pool.tile([P, KD * G], bf16)
                load_weight_bf16(wk_sb, wk_h, stage_pool)

                for chunk in range(T // TC):
                    t0 = chunk * TC
                    # natural x loads: tile bp holds rows (b=2bp..2bp+1, t0..t0+63)
                    natbs = []
                    for bp in range(4):
                        nat = nat_pool.tile([P, D], f32, name="nat", tag=f"nat{bp}")
                        for j in range(2):
                            nc.gpsimd.dma_start(
                                nat[j * TC:(j + 1) * TC, :],
                                x_ap[2 * bp + j, t0:t0 + TC, :],
                            )
                        natb = nat_pool.tile([P, D], bf16, name="natb", tag=f"natb{bp}")
                        nc.vector.tensor_copy(natb[:], nat[:])
                        natbs.append(natb)
                    # xbar transposes: xtb[k] cols = b*64 + t  (b-major)
                    xtbs = []
                    for k in range(KD):
                        xtb = xtb_pool.tile([P, TC * B_LOC], bf16,
                                            name=f"xtb{k}", tag=f"xtb{k}")
                        for bp in range(4):
                            nc.sync.dma_start(
                                xtb[:, bp * P:(bp + 1) * P],
                                natbs[bp][:, k * P:(k + 1) * P],
                                transpose=True,
                            )
                        xtbs.append(xtb)
                    for m in range(M):
                        ps = gps_pool.tile([P, TC * B_LOC], f32,
                                           name="gps", tag="gps")
                        for k in range(KD):
                            nc.tensor.matmul(
                                ps[:],
                                wk_sb[:, (k * M + m) * P:(k * M + m + 1) * P],
                                xtbs[k][:],
                                start=(k == 0),
                                stop=(k == KD - 1),
                            )
                        # copy-out + per-partition bias
                        # psum free = (b, t) b-major = contiguous dst slice
                        nc.scalar.activation(
                            zxT4[:, m, :, t0:t0 + TC],
                            ps.rearrange("p (b t) -> p b t", t=TC)[:],
                            AF.Identity,
                            bias=b_sb[:, PERMM[m]:PERMM[m] + 1],
                        )

            # ---------------- Phase B: the scan ----------------
            with (
                tc.tile_pool(name="wr", bufs=1) as wr_pool,
                tc.tile_pool(name="wstage2", bufs=2) as wstage2_pool,
                tc.tile_pool(name="state", bufs=1) as st_pool,
                tc.tile_pool(name="gates", bufs=2) as gate_pool,
                tc.tile_pool(name="tmp", bufs=2) as tmp_pool,
                tc.tile_pool(name="scan_psum", bufs=2, space="PSUM") as sps_pool,
            ):
                wr_sb = wr_pool.tile([P, KU * G], bf16)
                load_weight_bf16(wr_sb, wr_h, wstage2_pool)

                # h: bf16 per (parity, half); c: fp32 per (parity, half)
                hs = [[st_pool.tile([P, 2 * B_LOC], bf16, name=f"h{i}{j}")
                       for j in range(2)] for i in range(2)]
                cs = [[st_pool.tile([P, 2 * B_LOC], f32, name=f"c{i}{j}")
                       for j in range(2)] for i in range(2)]
                for j in range(2):
                    nc.vector.memset(hs[0][j][:], 0.0)
                    nc.vector.memset(cs[0][j][:], 0.0)
                hf = st_pool.tile([P, KU * B_LOC], f32, name="hf")

                # psum half tile col layout: a*16 + q*8 + b, a = gate class
                for t in range(time_steps):
                    pp = t % 2
                    qq = 1 - pp
                    h_prev = hs[pp]
                    pss = [sps_pool.tile([P, HB], f32, name=f"ps{hf_}",
                                         tag=f"ps{hf_}") for hf_ in range(2)]
                    # MM order: [half0 kk{0,1}], [half0 kk{2,3}],
                    #           [half1 kk{0,1}], [half1 kk{2,3}]
                    # - the first 16 pairs only need h half 0 (overlap with the
                    #   previous step's half-1 tail)
                    # - ps0 is complete after 32 pairs, so its tail starts at
                    #   the PE block's midpoint
                    # PSUM accumulation relies on per-element has_written:
                    # start=True only on the first MM per bank.
                    for half in range(2):
                        firstmm = True
                        for kpair in range(2):
                            for m in HALF_MS[half]:
                                a, q = m // 4, m % 4 - 2 * half
                                dst = pss[half][:, a * 16 + q * 8:
                                                a * 16 + q * 8 + 8]
                                for kk in (2 * kpair, 2 * kpair + 1):
                                    nc.tensor.matmul(
                                        dst,
                                        wr_sb[:, (kk * M + m) * P:
                                              (kk * M + m + 1) * P],
                                        h_prev[kk // 2][:, (kk % 2) * B_LOC:
                                                        (kk % 2 + 1) * B_LOC],
                                        start=firstmm,
                                        stop=(kpair == 1 and kk == KU - 1
                                              and m == HALF_MS[half][-1]),
                                        skip_group_check=True,
                                    )
                                    firstmm = False
                    last = t == time_steps - 1
                    prev_tc = None
                    prev_hmul = None
                    for half in range(2):
                        ps = pss[half]
                        # zx comb for this half: m = 4a + q + 2*half, all b,
                        # one t element each
                        zxh = (zxT4
                               .rearrange("p (a qq) b t -> p a qq b t", qq=4)
                               [:, :, 2 * half:2 * half + 2, :, t])
                        ps4 = ps.rearrange("p (a q b) -> p a q b", q=2, b=B_LOC)
                        i_zadd = nc.vector.tensor_add(ps4[:], ps4[:], zxh)
                        gt = gate_pool.tile([P, HB], f32, name=f"gt{half}",
                                            tag=f"gt{half}")
                        i_sig = nc.scalar.activation(gt[:, 0:48], ps[:, 0:48],
                                                     AF.Sigmoid)
                        nc.scalar.activation(gt[:, 48:64], ps[:, 48:64], AF.Tanh)
                        t1 = tmp_pool.tile([P, 2 * B_LOC], f32,
                                           name=f"t1{half}", tag=f"t1{half}")
                        nc.vector.tensor_mul(t1[:], gt[:, 16:32], cs[pp][half][:])
                        t2 = tmp_pool.tile([P, 2 * B_LOC], f32,
                                           name=f"t2{half}", tag=f"t2{half}")
                        nc.vector.tensor_mul(t2[:], gt[:, 0:16], gt[:, 48:64])
                        nc.vector.tensor_add(cs[qq][half][:], t1[:], t2[:])
                        tc_t = tmp_pool.tile([P, 2 * B_LOC], f32,
                                             name=f"tc{half}", tag=f"tc{half}")
                        i_tc = nc.scalar.activation(tc_t[:], cs[qq][half][:],
                                                    AF.Tanh)
                        if last:
                            i_hmul = nc.vector.tensor_mul(
                                hf[:, half * 16:(half + 1) * 16],
                                gt[:, 32:48], tc_t[:],
                            )
                        else:
                            i_hmul = nc.vector.tensor_mul(hs[qq][half][:],
                                                          gt[:, 32:48], tc_t[:])
                        if half == 1 and prev_tc is not None:
                            # keep ACT/DVE focused on the half-0 chain: half-1
                            # tail slots in only once half 0's h is produced
                            add_dep_helper(i_sig.ins, prev_tc.ins,
                                           reason="tail1 ACT after tail0 tanh_c")
                            add_dep_helper(i_zadd.ins, prev_hmul.ins,
                                           reason="tail1 zadd after tail0 h")
                        prev_tc, prev_hmul = i_tc, i_hmul

                for kk in range(KU):
                    nc.sync.dma_start(
                        out_h.ap()[:, kk * P:(kk + 1) * P].rearrange("b p -> p b"),
                        hf[:, kk * B_LOC:(kk + 1) * B_LOC],
                    )

    nc.compile()
    return nc


def _get_nc(time_steps=T):
    key = time_steps
    if key not in _CACHE:
        _CACHE[key] = _build(time_steps)
    return _CACHE[key]


def kernel(x, Wk, Wr, b):
    from concourse import bass_utils

    x = np.ascontiguousarray(np.asarray(x, dtype=np.float32))
    Wk = np.ascontiguousarray(np.asarray(Wk, dtype=np.float32))
    Wr = np.ascontiguousarray(np.asarray(Wr, dtype=np.float32))
    b = np.ascontiguousarray(np.asarray(b, dtype=np.float32))

    nc = _get_nc(T)
    in_maps = [
        {
            "x": x[c * B_LOC:(c + 1) * B_LOC],
            "Wk": Wk,
            "Wr": Wr,
            "b": b,
        }
        for c in range(N_CORES)
    ]
    res = bass_utils.run_bass_kernel_spmd(nc, in_maps, core_ids=list(range(N_CORES)))
    return np.concatenate([res.results[c]["h_last"] for c in range(N_CORES)], axis=0)



# revision 2
# speedup vs baseline: 1.3044x; 1.3044x over previous
"""Trainium2 Bass kernel for BasicLSTM (B=64, T=512, D=U=512).

Sharding: data-parallel over batch across 8 cores (8 rows/core), weights
replicated; the sequential time scan runs locally per core.

Per-core strategy (everything unit-major / "transposed", all-SBUF):
  Phase A: zx.T = Wk.T @ x.T + b computed directly in unit-major layout.
    x is loaded with fast contiguous DMAs, converted to bf16, transposed
    on-chip via the DMA xbar (dedicated queue), then used as the moving
    operand against stationary bf16 Wk tiles.  Bias is applied via the ACT
    per-partition bias during PSUM->SBUF copy-out.  The whole
    zx.T [128p, T*(16m*8b)] stays resident in SBUF as bf16 (16 MB).
    The g-gate columns of Wk/Wr/b are pre-scaled by 2 so the scan can
    evaluate all four gates with a single sigmoid (tanh(x) = 2*sigm(2x)-1).
  Phase B: 512-step scan with zero DMA.  Gate m-tiles are reordered
    [i,f,o,g]; the 16 m-tiles are split into two unit-halves, each with its
    own PSUM accumulator so the elementwise tail of one half overlaps the
    matmuls that wait on the other:
      psum[half] = zx.T[t] (identity matmul)        (PE, accumulate base)
                 + sum_k Wr[k,m].T @ h.T[k]         (PE, 64 LDW+MM pairs)
      s = sigmoid(psum)   (i,f,o and 2x-scaled g in one ACT instruction)
      t2 = (s_g - 0.5) * s_i ; c' = 2*t2 + f*c      (DVE fused STT x2,
                                                     f*c on Pool)
      h' = s_o * tanh(c')                           (ACT + DVE)
    The kk in {2,3} matmuls (needing only the first-finishing h half) are
    issued before the kk in {0,1} ones so the PE starts each step as soon
    as the first half of h(t-1) lands.  h is bf16 (feeds the next matmul),
    c stays fp32.  The final h is computed in fp32 and DMA'd out.
"""

import numpy as np

B, T, D, U = 64, 512, 512, 512
G = 4 * U            # gates
P = 128              # partitions
N_CORES = 8
B_LOC = B // N_CORES  # 8
KD = D // P          # 4 k-tiles for x@Wk
KU = U // P          # 4 k-tiles for h@Wr
M = G // P           # 16 m-tiles of gates
TC = 64              # timesteps per phase-A chunk
FB = M * B_LOC       # 128 free cols of z per step
HB = FB // 2         # 64 cols per half

# gate reordering: new m-tile order [i, f, o, g] -> original m-tile index
PERMM = list(range(8)) + [12, 13, 14, 15] + [8, 9, 10, 11]
# halves: half h holds m-tiles {4a + q : a in 0..3} for q in {2h, 2h+1}
HALF_MS = [[0, 4, 8, 12, 1, 5, 9, 13], [2, 6, 10, 14, 3, 7, 11, 15]]

_CACHE = {}


def _build(time_steps=T):
    import concourse.bacc as bacc
    import concourse.tile as tile
    import concourse.mybir as mybir
    from concourse import masks
    from concourse.alu_op_type import AluOpType

    f32 = mybir.dt.float32
    bf16 = mybir.dt.bfloat16
    AF = mybir.ActivationFunctionType

    nc = bacc.Bacc(
        "TRN2",
        target_bir_lowering=False,
        debug=False,
        enable_asserts=True,
        num_devices=N_CORES,
    )

    x_h = nc.dram_tensor("x", [B_LOC, T, D], f32, kind="ExternalInput")
    wk_h = nc.dram_tensor("Wk", [D, G], f32, kind="ExternalInput")
    wr_h = nc.dram_tensor("Wr", [U, G], f32, kind="ExternalInput")
    b_h = nc.dram_tensor("b", [G], f32, kind="ExternalInput")
    out_h = nc.dram_tensor("h_last", [B_LOC, U], f32, kind="ExternalOutput")

    x_ap = x_h.ap()

    def load_weight_bf16(dst, src_h, stage_pool):
        """[512, 2048] fp32 weight -> dst bf16 [128, 64*128] laid out as
        (k, new_m) tiles of [128, 128] with the [i,f,o,g] gate reorder.
        The g tiles (new m 12..15) are scaled by 2 for the sigmoid-only
        gate evaluation."""
        for k in range(KD):
            st = stage_pool.tile([P, G], f32, name="wstage", tag="wstage")
            nc.gpsimd.dma_start(st[:], src_h.ap()[k * P:(k + 1) * P, :])
            for nm0, om0, w in ((0, 0, 8), (8, 12, 4)):
                nc.vector.tensor_copy(
                    dst[:, (k * M + nm0) * P:(k * M + nm0 + w) * P],
                    st[:, om0 * P:(om0 + w) * P],
                )
            nc.vector.tensor_scalar_mul(
                dst[:, (k * M + 12) * P:(k * M + 16) * P],
                st[:, 8 * P:12 * P],
                2.0,
            )

    with tile.TileContext(nc) as tc:
        with (
            tc.tile_pool(name="persist", bufs=1) as persist_pool,
        ):
            # zx.T resident in SBUF: col = m*(T*8) + b*T + t  (bf16)
            zxT = persist_pool.tile([P, T * FB], bf16)
            zxT4 = zxT.rearrange("p (m b t) -> p m b t", m=M, b=B_LOC)
            # (a = gate class, q = global unit-block) view for the scan
            zxT5 = zxT.rearrange("p (a q b t) -> p a q b t", a=4, q=4, b=B_LOC)
            b_sb = persist_pool.tile([P, M], f32)
            nc.sync.dma_start(b_sb[:], b_h.ap().rearrange("(m p) -> p m", p=P))
            # double the g-gate bias (original m-tiles 8..11)
            nc.vector.tensor_scalar_mul(b_sb[:, 8:12], b_sb[:, 8:12], 2.0)
            ident = persist_pool.tile([P, P], bf16)
            masks.make_identity(nc, ident[:])

            # ---------------- Phase A: zx.T = Wk.T @ x.T + b ----------------
            with (
                tc.tile_pool(name="wk", bufs=1) as wk_pool,
                tc.tile_pool(name="stage", bufs=2) as stage_pool,
                tc.tile_pool(name="nat", bufs=2) as nat_pool,
                tc.tile_pool(name="xtb", bufs=2) as xtb_pool,
                tc.tile_pool(name="gemm_psum", bufs=4, space="PSUM") as gps_pool,
            ):
                wk_sb = wk_pool.tile([P, KD * G], bf16)
                load_weight_bf16(wk_sb, wk_h, stage_pool)

                for chunk in range(T // TC):
                    t0 = chunk * TC
                    # natural x loads: tile bp holds rows (b=2bp..2bp+1, t0..t0+63)
                    natbs = []
                    for bp in range(4):
                        nat = nat_pool.tile([P, D], f32, name="nat", tag=f"nat{bp}")
                        for j in range(2):
                            nc.gpsimd.dma_start(
                                nat[j * TC:(j + 1) * TC, :],
                                x_ap[2 * bp + j, t0:t0 + TC, :],
                            )
                        natb = nat_pool.tile([P, D], bf16, name="natb", tag=f"natb{bp}")
                        nc.vector.tensor_copy(natb[:], nat[:])
                        natbs.append(natb)
                    # xbar transposes: xtb[k] cols = b*64 + t  (b-major)
                    xtbs = []
                    for k in range(KD):
                        xtb = xtb_pool.tile([P, TC * B_LOC], bf16,
                                            name=f"xtb{k}", tag=f"xtb{k}")
                        for bp in range(4):
                            nc.sync.dma_start(
                                xtb[:, bp * P:(bp + 1) * P],
                                natbs[bp][:, k * P:(k + 1) * P],
                                transpose=True,
                            )
                        xtbs.append(xtb)
                    for m in range(M):
                        ps = gps_pool.tile([P, TC * B_LOC], f32,
                                           name="gps", tag="gps")
                        for k in range(KD):
                            nc.tensor.matmul(
                                ps[:],
                                wk_sb[:, (k * M + m) * P:(k * M + m + 1) * P],
                                xtbs[k][:],
                                start=(k == 0),
                                stop=(k == KD - 1),
                            )
                        # copy-out + per-partition bias
                        # psum free = (b, t) b-major = contiguous dst slice
                        nc.scalar.activation(
                            zxT4[:, m, :, t0:t0 + TC],
                            ps.rearrange("p (b t) -> p b t", t=TC)[:],
                            AF.Identity,
                            bias=b_sb[:, PERMM[m]:PERMM[m] + 1],
                        )

            # ---------------- Phase B: the scan ----------------
            with (
                tc.tile_pool(name="wr", bufs=1) as wr_pool,
                tc.tile_pool(name="wstage2", bufs=2) as wstage2_pool,
                tc.tile_pool(name="state", bufs=1) as st_pool,
                tc.tile_pool(name="gates", bufs=2) as gate_pool,
                tc.tile_pool(name="tmp", bufs=2) as tmp_pool,
                tc.tile_pool(name="scan_psum", bufs=2, space="PSUM") as sps_pool,
            ):
                wr_sb = wr_pool.tile([P, KU * G], bf16)
                load_weight_bf16(wr_sb, wr_h, wstage2_pool)

                # h: bf16 per (parity, half); c: fp32 per (parity, half)
                hs = [[st_pool.tile([P, 2 * B_LOC], bf16, name=f"h{i}{j}")
                       for j in range(2)] for i in range(2)]
                cs = [[st_pool.tile([P, 2 * B_LOC], f32, name=f"c{i}{j}")
                       for j in range(2)] for i in range(2)]
                for j in range(2):
                    nc.vector.memset(hs[0][j][:], 0.0)
                    nc.vector.memset(cs[0][j][:], 0.0)
                hf = st_pool.tile([P, KU * B_LOC], f32, name="hf")

                def mm_dst(ps, m, half):
                    a, q = m // 4, m % 4 - 2 * half
                    return ps[:, a * 16 + q * 8:a * 16 + q * 8 + 8]

                # psum half tile col layout: a*16 + q*8 + b, a = gate class
                for t in range(time_steps):
                    pp = t % 2
                    qq = 1 - pp
                    h_prev = hs[pp]
                    pss = [sps_pool.tile([P, HB], f32, name=f"ps{hf_}",
                                         tag=f"ps{hf_}") for hf_ in range(2)]
                    # zx lands in PSUM via an identity matmul (start=True);
                    # no input deps, so the PE runs these during the previous
                    # step's tail.
                    for half in range(2):
                        nc.tensor.matmul(
                            pss[half].rearrange("p (a q b) -> p a q b",
                                                a=4, q=2)[:],
                            ident[:],
                            zxT5[:, :, 2 * half:2 * half + 2, :, t],
                            start=True,
                            stop=False,
                            skip_group_check=True,
                        )
                    # Weight MMs accumulate on top (start=False).  kk 2,3
                    # need only h half 1 (finishes first); kk 0,1 need h
                    # half 0.  Within each kk group, the half-1 psum's tiles
                    # go first so its sigmoid can start a quarter early.
                    for kpair, gate_h in ((1, 1), (0, 0)):
                        kks = (2 * kpair, 2 * kpair + 1)
                        for half in (1, 0):
                            for m in HALF_MS[half]:
                                dst = mm_dst(pss[half], m, half)
                                for kk in kks:
                                    last = (kpair == 0 and kk == 1
                                            and m == HALF_MS[half][-1])
                                    nc.tensor.matmul(
                                        dst,
                                        wr_sb[:, (kk * M + m) * P:
                                              (kk * M + m + 1) * P],
                                        h_prev[kk // 2][:, (kk % 2) * B_LOC:
                                                        (kk % 2 + 1) * B_LOC],
                                        start=False,
                                        stop=last,
                                        skip_group_check=True,
                                    )
                    last_t = t == time_steps - 1
                    # tails; half 1's psum completes first, so its tail leads.
                    # ACT program order: sig1, sig0, tanh_c1, tanh_c0.
                    gts = [None, None]
                    t2s = [None, None]
                    t1s = [None, None]
                    for half in (1, 0):
                        gt = gate_pool.tile([P, HB], f32, name=f"gt{half}",
                                            tag=f"gt{half}")
                        # i,f,o sigmoids and tanh g = 2*sigm(2 z_g)-1 in one
                        # ACT op (g columns were pre-scaled by 2)
                        nc.scalar.activation(gt[:], pss[half][:], AF.Sigmoid)
                        gts[half] = gt
                    for half in (1, 0):
                        gt = gts[half]
                        t2 = tmp_pool.tile([P, 2 * B_LOC], f32,
                                           name=f"t2{half}", tag=f"t2{half}")
                        # t2 = (sig_g - 0.5) * i   [= i * tanh(g) / 2]
                        nc.vector.scalar_tensor_tensor(
                            t2[:], gt[:, 48:64], 0.5, gt[:, 0:16],
                            AluOpType.subtract, AluOpType.mult,
                        )
                        t1 = tmp_pool.tile([P, 2 * B_LOC], f32,
                                           name=f"t1{half}", tag=f"t1{half}")
                        nc.gpsimd.tensor_mul(t1[:], gt[:, 16:32], cs[pp][half][:])
                        # c' = 2*t2 + t1
                        nc.vector.scalar_tensor_tensor(
                            cs[qq][half][:], t2[:], 2.0, t1[:],
                            AluOpType.mult, AluOpType.add,
                        )
                        t2s[half], t1s[half] = t2, t1
                    tcs = [None, None]
                    for half in (1, 0):
                        tc_t = tmp_pool.tile([P, 2 * B_LOC], f32,
                                             name=f"tc{half}", tag=f"tc{half}")
                        nc.scalar.activation(tc_t[:], cs[qq][half][:], AF.Tanh)
                        tcs[half] = tc_t
                    for half in (1, 0):
                        if last_t:
                            nc.vector.tensor_mul(
                                hf[:, half * 16:(half + 1) * 16],
                                gts[half][:, 32:48], tcs[half][:],
                            )
                        else:
                            nc.vector.tensor_mul(hs[qq][half][:],
                                                 gts[half][:, 32:48],
                                                 tcs[half][:])

                for kk in range(KU):
                    nc.sync.dma_start(
                        out_h.ap()[:, kk * P:(kk + 1) * P].rearrange("b p -> p b"),
                        hf[:, kk * B_LOC:(kk + 1) * B_LOC],
                    )

    nc.compile()
    return nc


def _get_nc(time_steps=T):
    key = time_steps
    if key not in _CACHE:
        _CACHE[key] = _build(time_steps)
    return _CACHE[key]


def kernel(x, Wk, Wr, b):
    from concourse import bass_utils

    x = np.ascontiguousarray(np.asarray(x, dtype=np.float32))
    Wk = np.ascontiguousarray(np.asarray(Wk, dtype=np.float32))
    Wr = np.ascontiguousarray(np.asarray(Wr, dtype=np.float32))
    b = np.ascontiguousarray(np.asarray(b, dtype=np.float32))

    nc = _get_nc(T)
    in_maps = [
        {
            "x": x[c * B_LOC:(c + 1) * B_LOC],
            "Wk": Wk,
            "Wr": Wr,
            "b": b,
        }
        for c in range(N_CORES)
    ]
    res = bass_utils.run_bass_kernel_spmd(nc, in_maps, core_ids=list(range(N_CORES)))
    return np.concatenate([res.results[c]["h_last"] for c in range(N_CORES)], axis=0)
